# revision 8
# baseline (speedup 1.0000x reference)
"""AvgPool2d(64x64, stride 1, auto_pad-replicate) on TRN2, 8 NeuronCores.

Per (n, c) plane X [256, 256]:  inner = Bv^T @ X @ Bw  with banded 0/1
matrices [256, 193] (Bw carries 1/4096).  Two PE stages, no transposes:

Stage A (data stationary): y[w, io] = sum_h x[h, w] Bv[h, io].
  The band is only 64 wide, so each io column needs just ONE 128-row
  h-chunk pass: io in [0,64] lives in chunk0, [128,192] in chunk1, and
  [65,127] straddles - handled by accumulating both chunks' band slices
  (zeros in the band mask the out-of-window rows).  PSUM per-element
  has_written semantics (first touch overwrites, later touches add) let
  this run as just 2 matmuls of 128 cols per w-chunk: 4 MMs, 512
  streamed cols/plane (vs 8 MMs / 1544 cols for the naive 2-pass form).

Stage B (band stationary): o[jo, io] = sum_w Bw[w, jo] y[w, io].
  lhsT = Bw chunks are reused across a group of G planes, so the weight
  port sees ~0 load traffic here; 4 MMs x 193 cols per plane.  Output
  comes out transposed [jo, io], which the host undoes for free.

Evacuation: y [128,386] + oA [128,193] + oB [65,193] per plane, split
across DVE and ACT (GPSIMD has no PSUM port).  Output DMA is packed:
193 jo rows (128+65), not 256 - saves 24% of output bytes.

Sharding: pure data parallel, batch 16 -> 2 per core, 128 planes/core.
"""

import ml_dtypes
import numpy as np

import concourse.bass as bass
import concourse.tile as tile
from concourse import mybir
from concourse.bass_utils import run_bass_kernel_spmd


N_CORES = 8
N, C, H, W = 16, 64, 256, 256
KPOOL = 64
PLANES = (N // N_CORES) * C  # 128 per core
OUT_I = H - KPOOL + 1  # 193
PAD_LO = (H - OUT_I) // 2  # 31
PAD_HI = H - OUT_I - PAD_LO  # 32
OB = OUT_I - 128  # 65

MM_DT = mybir.dt.bfloat16
MM_NP = ml_dtypes.bfloat16

BATCH = 16  # planes per input DMA batch
G = 2  # stage-B pair size (two planes' y side by side, N = 386)
SLACK = 5  # stage-B for pair g issues after stage A of plane 2g+SLACK
N_WARM = 15  # HAM warmup matmuls on a memset tile (no DMA dependency)
SPLIT_BO = 0  # io-tail cols of the oB copy moved from ACT to DVE



def _band(n: int, k: int, scale: float) -> np.ndarray:
    """B[i, o] = scale if o <= i < o + k else 0;  [n, n-k+1]."""
    m = n - k + 1
    b = np.zeros((n, m), dtype=np.float32)
    for o in range(m):
        b[o : o + k, o] = scale
    return b


def _split_multiwaits(nc: bass.Bass) -> None:
    """Walrus codegen allows a single sync-wait slot per compute instruction.

    Tile's semaphore assignment can emit several; hoist the extras onto
    standalone NOPs in front of the instruction, on the same engine.
    """
    f = nc.m.functions[0]
    for block in f.blocks:
        out = []
        for inst in block.instructions:
            si = inst.sync_info
            if si is not None and len(si.on_wait) > 1:
                waits = list(si.on_wait)
                for w in waits[:-1]:
                    nop = mybir.InstNoOp(name=f"WS-{nc.next_id()}", ins=[], outs=[])
                    nop.engine = inst.engine
                    nop.sync_info = mybir.SyncInfo(on_wait=[w], on_update=[])
                    out.append(nop)
                inst.sync_info = mybir.SyncInfo(
                    on_wait=[waits[-1]], on_update=list(si.on_update)
                )
            out.append(inst)
        block.instructions = out


def _build(split_waits: bool = True) -> bass.Bass:
    nc = bass.Bass()
    # partition-major layouts: x [r, plane, k, w]
    x_ext = nc.declare_dram_parameter(
        "x", [128, PLANES, 2, W], MM_DT, isOutput=False
    )
    bv_ext = nc.declare_dram_parameter("bv", [H, OUT_I], MM_DT, isOutput=False)
    bw_ext = nc.declare_dram_parameter("bw", [W, OUT_I], MM_DT, isOutput=False)
    # packed outputs, transposed per plane: outA[r, p, io] = inner[p, io, jo=r],
    # outB[r, p, io] = inner[p, io, jo=128+r]
    outA_ext = nc.declare_dram_parameter(
        "outA", [128, PLANES, OUT_I], MM_DT, isOutput=True
    )
    outB_ext = nc.declare_dram_parameter(
        "outB", [OB, PLANES, OUT_I], MM_DT, isOutput=True
    )

    n_batches = PLANES // BATCH

    with tile.TileContext(nc) as tc:
        with (
            tc.tile_pool(name="consts", bufs=1) as consts,
            tc.tile_pool(name="xin", bufs=4) as xpool,
            tc.tile_pool(name="ysb", bufs=10) as ypool_sb,
            tc.tile_pool(name="osb", bufs=2) as opool_sb,
            tc.tile_pool(name="yps", bufs=2, space="PSUM") as ypool_ps,
            tc.tile_pool(name="ops", bufs=2, space="PSUM") as opool_ps,
        ):
            # Warmup source: memset tile, no DMA dependency, so the PE can
            # start warming the HAM clock gate during the DMA head.
            warm_sb = consts.tile([128, 256], MM_DT)
            nc.vector.memset(warm_sb, 0.0)

            x_tiles = [None] * n_batches
            o_sb = [None] * n_batches
            y_tiles = {}

            # Input stream: first plane first, then the band consts, then the
            # rest - so plane 0 and the consts land as early as possible.
            x_tiles[0] = xpool.tile([128, BATCH, 2, W], MM_DT, name="x_sb")
            nc.sync.dma_start(out=x_tiles[0][:, 0:1], in_=x_ext[:, 0:1, :, :])
            bv_sb = consts.tile([128, 2, OUT_I], MM_DT)
            nc.scalar.dma_start(
                out=bv_sb, in_=bv_ext[:, :].rearrange("(k r) o -> r k o", k=2)
            )
            bw_sb = consts.tile([128, 2, OUT_I], MM_DT)
            nc.scalar.dma_start(
                out=bw_sb, in_=bw_ext[:, :].rearrange("(k r) o -> r k o", k=2)
            )
            for lo, hi in ((1, 2), (2, 4), (4, 8), (8, 16)):
                nc.sync.dma_start(
                    out=x_tiles[0][:, lo:hi], in_=x_ext[:, lo:hi, :, :]
                )

            def dma_in(b):
                x_tiles[b] = xpool.tile([128, BATCH, 2, W], MM_DT, name="x_sb")
                for lo, hi in ((0, 8), (8, 16)):
                    nc.sync.dma_start(
                        out=x_tiles[b][:, lo:hi],
                        in_=x_ext[:, b * BATCH + lo : b * BATCH + hi, :, :],
                    )

            warm_ps = opool_ps.tile(
                [128, 1024], mybir.dt.float32, name="warm_ps", tag="o_ps"
            )
            for _ in range(N_WARM):
                nc.tensor.matmul(
                    warm_ps[:, 0:OUT_I],
                    lhsT=warm_sb[:, 0:128],
                    rhs=warm_sb[:, 0:OUT_I],
                    start=True,
                    stop=True,
                )

            y_ps_cur = [None]

            def stage_a(i):
                b, p = divmod(i, BATCH)
                if p == 0 and b > 0:
                    dma_in(b)
                x_sb = x_tiles[b]
                j = i % G
                if j == 0:
                    # pair PSUM tile: plane j owns bank j (cols [512j, 512j+386))
                    y_ps_cur[0] = ypool_ps.tile(
                        [128, 1024], mybir.dt.float32, name="y_ps"
                    )
                y_ps = y_ps_cur[0]
                base = 512 * j
                # Merged band passes: MM1 streams all 193 io cols of the
                # chunk0 band rows (zeros beyond the band write 0s + set
                # has_written), MM2 accumulates the chunk1 rows for io>=65.
                # start=True clears the whole BANK's has_written bits, so
                # each m's accumulate completes before the next start.
                for m in range(2):
                    nc.tensor.matmul(
                        y_ps[:, base + m * OUT_I : base + (m + 1) * OUT_I],
                        lhsT=x_sb[:, p, 0, m * 128 : (m + 1) * 128],
                        rhs=bv_sb[:, 0, :],
                        start=True,
                        stop=False,
                        skip_group_check=True,
                    )
                    nc.tensor.matmul(
                        y_ps[:, base + m * OUT_I + 65 : base + (m + 1) * OUT_I],
                        lhsT=x_sb[:, p, 1, m * 128 : (m + 1) * 128],
                        rhs=bv_sb[:, 1, 65:OUT_I],
                        start=False,
                        stop=(m == 1),
                        skip_group_check=True,
                    )
                if j == G - 1:
                    # one DVE copy per pair; y_sb [128, plane, chunk, io]
                    y_sb = ypool_sb.tile([128, G, 2, OUT_I], MM_DT, name="y_sb")
                    nc.vector.tensor_copy(
                        y_sb,
                        y_ps.rearrange("r (j c) -> r j c", j=G)[:, :, 0 : 2 * OUT_I],
                    )
                    y_tiles[i // G] = y_sb

            def stage_b(g):
                q0 = G * g
                b, p0 = divmod(q0, BATCH)
                if p0 == 0:
                    o_sb[b] = opool_sb.tile(
                        [128, 2, BATCH, OUT_I], MM_DT, name="o_sb"
                    )
                yp = y_tiles.pop(g)
                # pair PSUM tile: bank0 = oA [128, 386], bank1 = oB [65, 386]
                o_ps = opool_ps.tile(
                    [128, 1024], mybir.dt.float32, name="o_ps", tag="o_ps"
                )
                # 3 wide matmuls per plane pair (N = 2*193), band stationary:
                # jo [0,127] from w chunk0 (band zeros mask rows < jo-63),
                # jo [128,192] from chunk1, then jo [64,127] chunk1 accumulate.
                nc.tensor.matmul(
                    o_ps[:, 0 : G * OUT_I],
                    lhsT=bw_sb[:, 0, 0:128],
                    rhs=yp[:, :, 0, :],
                    start=True,
                    stop=False,
                    skip_group_check=True,
                )
                nc.tensor.matmul(
                    o_ps[64:128, 0 : G * OUT_I],
                    lhsT=bw_sb[:, 1, 64:128],
                    rhs=yp[:, :, 1, :],
                    start=False,
                    stop=True,
                    skip_group_check=True,
                )
                nc.tensor.matmul(
                    o_ps[0:OB, 512 : 512 + G * OUT_I],
                    lhsT=bw_sb[:, 1, 128:OUT_I],
                    rhs=yp[:, :, 1, :],
                    start=True,
                    stop=True,
                    skip_group_check=True,
                )
                # one ACT copy per pair: [kind, plane, io] <- both banks
                nc.scalar.copy(
                    o_sb[b][:, :, p0 : p0 + G, :],
                    o_ps.rearrange("r (k c) -> r k c", k=2)[:, :, 0 : G * OUT_I],
                )
                p = p0 + G - 1
                flush = (3, 7, 11, 13, 15) if b == n_batches - 1 else (7, 15)
                if p in flush:
                    prev = ([-1] + list(flush))[flush.index(p)] + 1
                    ring = (b + flush.index(p)) % 2
                    engA = nc.scalar if ring == 0 else nc.sync
                    engB = nc.sync if ring == 0 else nc.scalar
                    engA.dma_start(
                        out=outA_ext[:, b * BATCH + prev : b * BATCH + p + 1, :],
                        in_=o_sb[b][:, 0, prev : p + 1, :],
                    )
                    engB.dma_start(
                        out=outB_ext[:, b * BATCH + prev : b * BATCH + p + 1, :],
                        in_=o_sb[b][0:OB, 1, prev : p + 1, :],
                    )

            n_groups = PLANES // G
            done_b = 0
            for i in range(PLANES):
                stage_a(i)
                if i >= SLACK and (i - SLACK) % G == G - 1:
                    stage_b((i - SLACK) // G)
                    done_b += 1
            for g in range(done_b, n_groups):
                stage_b(g)

    if split_waits:
        _split_multiwaits(nc)
    return nc


_NC_CACHE = None


def _get_nc():
    global _NC_CACHE
    if _NC_CACHE is None:
        _NC_CACHE = _build()
    return _NC_CACHE


def _run(x: np.ndarray, trace: bool = False):
    x = np.asarray(x, dtype=np.float32)
    assert x.shape == (N, C, H, W), x.shape
    # partition-major repack: [core, plane, (k r), w] -> [core, r, plane, k, w]
    xs = x.reshape(N_CORES, PLANES, 2, 128, W).transpose(0, 3, 1, 2, 4)
    xs = np.ascontiguousarray(xs, dtype=np.float32).astype(MM_NP)
    bv = _band(H, KPOOL, 1.0).astype(MM_NP)
    bw = _band(W, KPOOL, 1.0 / (KPOOL * KPOOL)).astype(MM_NP)
    in_maps = [{"x": xs[i], "bv": bv, "bw": bw} for i in range(N_CORES)]
    # The device sporadically reports NRT_EXEC_UNIT_UNRECOVERABLE even for a
    # known-good NEFF; retry a couple of times before giving up.
    last_err = None
    for attempt in range(3):
        try:
            res = run_bass_kernel_spmd(
                nc=_get_nc(),
                in_maps=in_maps,
                core_ids=list(range(N_CORES)),
                trace=trace,
            )
            break
        except Exception as e:  # noqa: BLE001
            last_err = e
            import time

            time.sleep(2.0 * (attempt + 1))
    else:
        raise last_err
    outs = []
    for i in range(N_CORES):
        oA = np.asarray(res.results[i]["outA"], dtype=np.float32)  # [128, p, io]
        oB = np.asarray(res.results[i]["outB"], dtype=np.float32)  # [65, p, io]
        # inner[p, io, jo]
        inner = np.concatenate(
            [oA.transpose(1, 2, 0), oB.transpose(1, 2, 0)], axis=2
        )
        outs.append(inner)
    inner = np.stack(outs, axis=0)  # [cores, planes, 193, 193]
    full = np.pad(
        inner, ((0, 0), (0, 0), (PAD_LO, PAD_HI), (PAD_LO, PAD_HI)), mode="edge"
    )
    return full.reshape(N, C, H, W), res


def kernel(x: np.ndarray) -> np.ndarray:
    out, _ = _run(x, trace=False)
    return out
